# revision 17
# baseline (speedup 1.0000x reference)
"""Single-head causal attention (B=8, T=2048, D=1024, H=128) on 8 TRN2
NeuronCores — data-parallel over batch (one batch element per core).

x is uploaded pre-transposed (host-side layout permutation, dtype
preserved f32): device DRAM holds x^T as [q-chunk, d-tile, p, t-chunk]
blocks, so the kernel spends zero TensorE time on transposes and goes
straight to projections. All reference math (projections, scores,
softmax, PV) runs on device in bf16 with f32 accumulation.

Per-core dataflow:
  0. All DMAs issue first: wq + 32 x^T blocks (256 KB each, chunk-major)
     + output stores on the sync HWDGE ring (a trigger blocks its issuing
     sequencer once the ring backs up; sync has no compute to starve),
     wk/wv on the scalar ring. Weights host-reshaped to [p, dt, h].
  1. PE warmup matmuls (single tile, same-engine WAW: no semaphores)
     flip the HAM clock-gate to 2.4 GHz during the x-arrival window.
  2. Per q-chunk c: DVE-cast the 8 arriving f32 blocks into bf16
     xT [d-part, d-tile, t]; project qT/kT/vT with N=512 matmuls
     (weights stationary); PE-transpose vT tiles into v_aug
     [t-tile, 129] (v natural + ones column that makes PV also produce
     the softmax denominator). Next chunk's casts are emitted before
     this chunk's attention so the DVE FIFO never parks them behind the
     divides.
  3. Attention for chunk c is emitted right after its projections, so
     exp on ScalarE overlaps the next chunk's projections on TensorE.
     Scores TRANSPOSED per k-tile: ST[k 128, q 512] = kT_tile^T @
     qT_chunk, one PSUM bank each, pool-double-buffered with a one-tile
     lookahead. Causality: lower-left tiles skipped, diagonal tiles exp
     only the valid range and zero the 128x128 triangle via GpSimd
     affine_select on PT.
  4. O[q 128, 129] += PT_slice^T @ v_aug_tile accumulated over k-tiles in
     PSUM (one bank per q-tile — start=True clears has_written bank-wide,
     so accumulators must not share banks); col 128 is the softmax
     denominator. Each q-tile's divide + DMA out fire at its chain stop.

  Hot-path tiles come from pools (tag + bufs): a pool slot hand-off
  aggregates all readers into one release edge, while direct tile reuse
  makes every writer wait on each prior reader (~15% kernel-wide
  inflation when tried).
"""

import numpy as np

import concourse.bass as bass
import concourse.bacc as bacc
import concourse.mybir as mybir
import concourse.tile as tile
from concourse import bass_utils
from concourse.masks import make_identity

B, T, D, H = 8, 2048, 1024, 128
P = 128
DT = D // P  # 8 d tiles
TT = T // P  # 16 t tiles
CH = 512  # q chunk width
QC = T // CH  # 4 q chunks
N_CORES = 8
SCALE = float(1.0 / np.sqrt(H))
N_WARMUP = 30

F32 = mybir.dt.float32
BF16 = mybir.dt.bfloat16


def build_nc():
    nc = bacc.Bacc("TRN2", target_bir_lowering=False, debug=False)
    # x^T as t-tile slabs [t-tile, p(d), dt, u(t)], contiguous 4KB/partition
    xt_d = nc.dram_tensor("xt", [TT, P, DT, P], F32, kind="ExternalInput").ap()
    wq_d = nc.dram_tensor("wq", [P, DT, H], F32, kind="ExternalInput").ap()
    wk_d = nc.dram_tensor("wk", [P, DT, H], F32, kind="ExternalInput").ap()
    wv_d = nc.dram_tensor("wv", [P, DT, H], F32, kind="ExternalInput").ap()
    out = nc.dram_tensor("out", [T, H], F32, kind="ExternalOutput").ap()

    with tile.TileContext(nc) as tc:
        _build_body(nc, tc, xt_d, wq_d, wk_d, wv_d, out)
    nc.compile()
    return nc


def _build_body(nc, tc, xt_d, wq_d, wk_d, wv_d, out):
    with (
        tc.tile_pool(name="persist", bufs=1) as persist,
        tc.tile_pool(name="xpool", bufs=TT) as xpool,
        tc.tile_pool(name="work", bufs=4) as work,
        tc.tile_pool(name="ps", bufs=1, space="PSUM") as ps,
    ):
        # ---- all DMAs first ----
        xs = [
            xpool.tile([P, DT, P], F32, tag="xs", name=f"xs{t}") for t in range(TT)
        ]
        wf = {
            nm: work.tile([P, DT, H], F32, tag="wf32", name=f"{nm}_f32")
            for nm in ("wq", "wk", "wv")
        }
        nc.sync.dma_start(wf["wq"][:], wq_d)
        for t in range(TT):
            nc.sync.dma_start(xs[t][:], xt_d[t])
        nc.scalar.dma_start(wf["wk"][:], wk_d)
        nc.scalar.dma_start(wf["wv"][:], wv_d)

        # ---- constants ----
        ident_b = persist.tile([P, P], BF16, tag="ident_b", name="ident_b")
        make_identity(nc, ident_b)

        v_aug = persist.tile([P, TT, H + 1], BF16, tag="v_aug", name="v_aug")
        nc.gpsimd.memset(v_aug[:, :, H : H + 1], 1.0)  # ones col (denominator)

        warm_src = persist.tile([P, 3 * P], BF16, tag="warm_src", name="warm_src")
        nc.vector.memset(warm_src[:], 0.5)

        warm_ps = ps.tile([P, 3 * P], F32, tag="st", bufs=2, name="warm_ps")

        def warm(n):
            for _ in range(n):
                nc.tensor.matmul(
                    warm_ps[:], ident_b[:], warm_src[:], start=True, stop=True
                )

        warm(N_WARMUP)

        # weights cast to bf16 on DVE
        w_bf = []
        for nm in ("wq", "wk", "wv"):
            wb = persist.tile([P, DT, H], BF16, tag=f"{nm}b", name=f"{nm}_bf")
            nc.vector.tensor_copy(wb[:], wf[nm][:])
            w_bf.append(wb)
        wq_bf, wk_bf, wv_bf = w_bf

        # ---- persistent activations ----
        xT = persist.tile([P, DT, T], BF16, tag="xT", name="xT")
        qT = persist.tile([P, T], BF16, tag="qT", name="qT")
        kT = persist.tile([P, T], BF16, tag="kT", name="kT")
        vT = persist.tile([P, T], BF16, tag="vT", name="vT")

        def emit_casts(c):
            # f32 slabs -> bf16 xT columns for chunk c (DVE, 2x mode)
            for t in range(4 * c, 4 * c + 4):
                nc.vector.tensor_copy(xT[:, :, t * P : (t + 1) * P], xs[t][:])

        emit_casts(0)
        for c in range(QC):
            # ---- projections for this chunk, weights stationary.
            # q of chunk 0 goes in N=128 pieces chasing slab arrival (only
            # the first piece carries start=True: it clears the whole
            # bank's has_written; later pieces overwrite-where-clear /
            # accumulate-where-set per element). Warm matmuls fill the
            # arrival gaps so HAM stays at 2.4 GHz. ----
            t0 = c * CH
            proj = [("q", wq_bf, qT), ("k", wk_bf, kT)]
            prv = ("v", wv_bf, vT)
            pr_out = {}
            for nm, wb, dstT in proj + [prv]:
                pr_out[nm] = ps.tile(
                    [P, CH], F32, tag="mm", bufs=2, name=f"{nm}T_ps{c}"
                )

            def do_proj(nm, wb, dstT, pieces, c=c, t0=t0, pr_out=pr_out):
                pr_ps = pr_out[nm]
                npc = 4 if pieces else 1
                w = CH // npc
                for ts in range(npc):
                    for dt in range(DT):
                        nc.tensor.matmul(
                            pr_ps[:, ts * w : (ts + 1) * w],
                            wb[:, dt, :],
                            xT[:, dt, t0 + ts * w : t0 + (ts + 1) * w],
                            start=(ts == 0 and dt == 0),
                            stop=(ts == npc - 1 and dt == DT - 1),
                        )
                if nm == "v":
                    nc.scalar.copy(dstT[:, t0 : t0 + CH], pr_ps[:])
                else:
                    nc.vector.tensor_copy(dstT[:, t0 : t0 + CH], pr_ps[:])

            for nm, wb, dstT in proj:
                do_proj(nm, wb, dstT, pieces=(c == 0 and nm == "q"))

            # ---- attention setup: first two score tiles go ahead of the
            # v-projection so ScalarE exp starts as early as possible ----
            n_k = 4 * c + 4  # k-tiles 0..4c+3
            o_ps = [
                ps.tile([P, H + 1], F32, tag="o", bufs=4, name=f"o{c}_{s}")
                for s in range(4)
            ]
            st_ps = {}

            def emit_s(i, c=c, st_ps=st_ps):
                st = ps.tile([P, CH], F32, tag="st", bufs=2, name=f"st{c}_{i}")
                e0 = max(i - 4 * c, 0) * P
                nc.tensor.matmul(
                    st[:, e0:CH],
                    kT[:, i * P : (i + 1) * P],
                    qT[:, c * CH + e0 : (c + 1) * CH],
                    start=True,
                    stop=True,
                )
                st_ps[i] = st

            emit_s(0)
            if n_k > 1:
                emit_s(1)

            do_proj(*prv, pieces=False)
            # v natural tiles: PE-transpose vT tiles into v_aug
            for tt in range(4 * c, 4 * c + 4):
                vtr = ps.tile([P, P], BF16, tag="mm", bufs=2, name=f"vtr{tt}")
                nc.tensor.transpose(vtr[:], vT[:, tt * P : (tt + 1) * P], ident_b)
                nc.vector.tensor_copy(v_aug[:, tt, 0:H], vtr[:])
            # next chunk's casts go to the DVE queue ahead of the divides
            if c + 1 < QC:
                emit_casts(c + 1)

            for i in range(n_k):
                if i + 2 < n_k:
                    emit_s(i + 2)  # keep PE fed while ACT does exp(i)
                st = st_ps.pop(i)
                pt = work.tile([P, CH], BF16, tag="pt", name=f"pt{c}_{i}")
                j0 = i - 4 * c  # diag offset (>=0 on the diagonal tile)
                e0 = max(j0, 0) * P
                nc.scalar.activation(
                    pt[:, e0:CH],
                    st[:, e0:CH],
                    mybir.ActivationFunctionType.Exp,
                    scale=SCALE,
                )
                if j0 >= 0:
                    # zero the causal triangle of the diagonal block
                    nc.gpsimd.affine_select(
                        out=pt[:, e0 : e0 + P],
                        in_=pt[:, e0 : e0 + P],
                        compare_op=mybir.AluOpType.is_ge,
                        fill=0.0,
                        base=0,
                        pattern=[[1, P]],
                        channel_multiplier=-1,
                    )
                for s in range(4):
                    if i <= 4 * c + s:
                        nc.tensor.matmul(
                            o_ps[s][:],
                            pt[:, s * P : (s + 1) * P],
                            v_aug[:, i, :],
                            start=(i == 0),
                            stop=(i == 4 * c + s),
                        )
                # q-tile whose accumulation chain just stopped: divide + out
                s = i - 4 * c
                if 0 <= s < 4:
                    qt_idx = 4 * c + s
                    recip = work.tile(
                        [P, 1], F32, tag="recip", bufs=16, name=f"rcp{qt_idx}"
                    )
                    nc.vector.reciprocal(recip[:], o_ps[s][:, H : H + 1])
                    o_sb = work.tile(
                        [P, H], F32, tag="o_sb", bufs=16, name=f"o_sb{qt_idx}"
                    )
                    nc.vector.tensor_scalar_mul(o_sb[:], o_ps[s][:, 0:H], recip[:])
                    nc.sync.dma_start(out[qt_idx * P : (qt_idx + 1) * P, :], o_sb[:])


_NC_CACHE = None


def _get_nc():
    global _NC_CACHE
    if _NC_CACHE is None:
        _NC_CACHE = build_nc()
    return _NC_CACHE


def kernel(**inputs):
    x = np.asarray(inputs["x"], dtype=np.float32)

    def host_reshape(w):
        # [D, H] -> [p, dt, h] (pure layout permutation, dtype preserved)
        w = np.asarray(w, dtype=np.float32)
        return np.ascontiguousarray(w.reshape(DT, P, H).transpose(1, 0, 2))

    wq = host_reshape(inputs["Wq"])
    wk = host_reshape(inputs["Wk"])
    wv = host_reshape(inputs["Wv"])
    assert x.shape == (B, T, D)

    def host_xt(xb):
        # [T, D] -> x^T t-tile slabs [tt, p, dt, u] (layout permutation only)
        return np.ascontiguousarray(
            xb.T.reshape(DT, P, TT, P).transpose(2, 1, 0, 3)
        )

    nc = _get_nc()
    in_maps = [
        {"xt": host_xt(x[b]), "wq": wq, "wk": wk, "wv": wv} for b in range(N_CORES)
    ]
    res = bass_utils.run_bass_kernel_spmd(nc, in_maps, core_ids=list(range(N_CORES)))
    return np.stack([res.results[b]["out"] for b in range(N_CORES)], axis=0)


# revision 18
# speedup vs baseline: 1.0231x; 1.0231x over previous
"""Single-head causal attention (B=8, T=2048, D=1024, H=128) on 8 TRN2
NeuronCores — data-parallel over batch (one batch element per core).

x is uploaded pre-transposed (host-side layout permutation, dtype
preserved f32): device DRAM holds x^T as [q-chunk, d-tile, p, t-chunk]
blocks, so the kernel spends zero TensorE time on transposes and goes
straight to projections. All reference math (projections, scores,
softmax, PV) runs on device in bf16 with f32 accumulation.

Per-core dataflow:
  0. All DMAs issue first: wq + 32 x^T blocks (256 KB each, chunk-major)
     + output stores on the sync HWDGE ring (a trigger blocks its issuing
     sequencer once the ring backs up; sync has no compute to starve),
     wk/wv on the scalar ring. Weights host-reshaped to [p, dt, h].
  1. PE warmup matmuls (single tile, same-engine WAW: no semaphores)
     flip the HAM clock-gate to 2.4 GHz during the x-arrival window.
  2. Per q-chunk c: DVE-cast the 8 arriving f32 blocks into bf16
     xT [d-part, d-tile, t]; project qT/kT/vT with N=512 matmuls
     (weights stationary); PE-transpose vT tiles into v_aug
     [t-tile, 129] (v natural + ones column that makes PV also produce
     the softmax denominator). Next chunk's casts are emitted before
     this chunk's attention so the DVE FIFO never parks them behind the
     divides.
  3. Attention for chunk c is emitted right after its projections, so
     exp on ScalarE overlaps the next chunk's projections on TensorE.
     Scores TRANSPOSED per k-tile: ST[k 128, q 512] = kT_tile^T @
     qT_chunk, one PSUM bank each, pool-double-buffered with a one-tile
     lookahead. Causality: lower-left tiles skipped, diagonal tiles exp
     only the valid range and zero the 128x128 triangle via GpSimd
     affine_select on PT.
  4. O[q 128, 129] += PT_slice^T @ v_aug_tile accumulated over k-tiles in
     PSUM (one bank per q-tile — start=True clears has_written bank-wide,
     so accumulators must not share banks); col 128 is the softmax
     denominator. Each q-tile's divide + DMA out fire at its chain stop.

  Hot-path tiles come from pools (tag + bufs): a pool slot hand-off
  aggregates all readers into one release edge, while direct tile reuse
  makes every writer wait on each prior reader (~15% kernel-wide
  inflation when tried).
"""

import numpy as np

import concourse.bass as bass
import concourse.bacc as bacc
import concourse.mybir as mybir
import concourse.tile as tile
from concourse import bass_utils
from concourse.masks import make_identity

B, T, D, H = 8, 2048, 1024, 128
P = 128
DT = D // P  # 8 d tiles
TT = T // P  # 16 t tiles
CH = 512  # q chunk width
QC = T // CH  # 4 q chunks
N_CORES = 8
SCALE = float(1.0 / np.sqrt(H))
N_WARMUP = 22

F32 = mybir.dt.float32
BF16 = mybir.dt.bfloat16


def build_nc():
    nc = bacc.Bacc("TRN2", target_bir_lowering=False, debug=False)
    # x^T as t-tile slabs [t-tile, p(d), dt, u(t)], contiguous 4KB/partition
    xt_d = nc.dram_tensor("xt", [TT, P, DT, P], F32, kind="ExternalInput").ap()
    wq_d = nc.dram_tensor("wq", [P, DT, H], F32, kind="ExternalInput").ap()
    wk_d = nc.dram_tensor("wk", [P, DT, H], F32, kind="ExternalInput").ap()
    wv_d = nc.dram_tensor("wv", [P, DT, H], F32, kind="ExternalInput").ap()
    out = nc.dram_tensor("out", [T, H], F32, kind="ExternalOutput").ap()

    with tile.TileContext(nc) as tc:
        _build_body(nc, tc, xt_d, wq_d, wk_d, wv_d, out)
    nc.compile()
    return nc


def _build_body(nc, tc, xt_d, wq_d, wk_d, wv_d, out):
    with (
        tc.tile_pool(name="persist", bufs=1) as persist,
        tc.tile_pool(name="xpool", bufs=TT) as xpool,
        tc.tile_pool(name="work", bufs=4) as work,
        tc.tile_pool(name="ps", bufs=1, space="PSUM") as ps,
    ):
        # ---- all DMAs first ----
        xs = [
            xpool.tile([P, DT, P], F32, tag="xs", name=f"xs{t}") for t in range(TT)
        ]
        wf = {
            nm: work.tile([P, DT, H], F32, tag="wf32", name=f"{nm}_f32")
            for nm in ("wq", "wk", "wv")
        }
        nc.sync.dma_start(wf["wq"][:], wq_d)
        for t in range(TT):
            nc.sync.dma_start(xs[t][:], xt_d[t])
        nc.scalar.dma_start(wf["wk"][:], wk_d)
        nc.scalar.dma_start(wf["wv"][:], wv_d)

        # ---- constants ----
        ident_b = persist.tile([P, P], BF16, tag="ident_b", name="ident_b")
        make_identity(nc, ident_b)

        v_aug = persist.tile([P, TT, H + 1], BF16, tag="v_aug", name="v_aug")
        nc.gpsimd.memset(v_aug[:, :, H : H + 1], 1.0)  # ones col (denominator)

        warm_src = persist.tile([P, 3 * P], BF16, tag="warm_src", name="warm_src")
        nc.vector.memset(warm_src[:], 0.5)

        warm_ps = ps.tile([P, 3 * P], F32, tag="st", bufs=2, name="warm_ps")

        def warm(n):
            for _ in range(n):
                nc.tensor.matmul(
                    warm_ps[:], ident_b[:], warm_src[:], start=True, stop=True
                )

        warm(N_WARMUP)

        # weights cast to bf16 on DVE
        w_bf = []
        for nm in ("wq", "wk", "wv"):
            wb = persist.tile([P, DT, H], BF16, tag=f"{nm}b", name=f"{nm}_bf")
            nc.vector.tensor_copy(wb[:], wf[nm][:])
            w_bf.append(wb)
        wq_bf, wk_bf, wv_bf = w_bf

        # ---- persistent activations ----
        xT = persist.tile([P, DT, T], BF16, tag="xT", name="xT")
        qT = persist.tile([P, T], BF16, tag="qT", name="qT")
        kT = persist.tile([P, T], BF16, tag="kT", name="kT")
        vT = persist.tile([P, T], BF16, tag="vT", name="vT")

        def emit_casts(c):
            # f32 slabs -> bf16 xT columns for chunk c (DVE, 2x mode)
            for t in range(4 * c, 4 * c + 4):
                nc.vector.tensor_copy(xT[:, :, t * P : (t + 1) * P], xs[t][:])

        emit_casts(0)
        for c in range(QC):
            # ---- projections for this chunk, weights stationary.
            # q of chunk 0 goes in N=128 pieces chasing slab arrival (only
            # the first piece carries start=True: it clears the whole
            # bank's has_written; later pieces overwrite-where-clear /
            # accumulate-where-set per element). Warm matmuls fill the
            # arrival gaps so HAM stays at 2.4 GHz. ----
            t0 = c * CH
            proj = [("q", wq_bf, qT), ("k", wk_bf, kT)]
            prv = ("v", wv_bf, vT)
            pr_out = {}
            for nm, wb, dstT in proj + [prv]:
                pr_out[nm] = ps.tile(
                    [P, CH], F32, tag="mm", bufs=2, name=f"{nm}T_ps{c}"
                )

            def do_proj(nm, wb, dstT, pieces, c=c, t0=t0, pr_out=pr_out):
                pr_ps = pr_out[nm]
                npc = 4 if pieces else 1
                w = CH // npc
                for ts in range(npc):
                    for dt in range(DT):
                        nc.tensor.matmul(
                            pr_ps[:, ts * w : (ts + 1) * w],
                            wb[:, dt, :],
                            xT[:, dt, t0 + ts * w : t0 + (ts + 1) * w],
                            start=(ts == 0 and dt == 0),
                            stop=(ts == npc - 1 and dt == DT - 1),
                        )
                    if pieces:
                        warm(2)  # fill slab-arrival gaps, keep HAM hot
                if nm == "v":
                    nc.scalar.copy(dstT[:, t0 : t0 + CH], pr_ps[:])
                else:
                    nc.vector.tensor_copy(dstT[:, t0 : t0 + CH], pr_ps[:])

            for nm, wb, dstT in proj:
                do_proj(nm, wb, dstT, pieces=(c == 0 and nm == "q"))

            # ---- attention setup: first two score tiles go ahead of the
            # v-projection so ScalarE exp starts as early as possible ----
            n_k = 4 * c + 4  # k-tiles 0..4c+3
            o_ps = [
                ps.tile([P, H + 1], F32, tag="o", bufs=4, name=f"o{c}_{s}")
                for s in range(4)
            ]
            st_ps = {}

            def emit_s(i, c=c, st_ps=st_ps):
                st = ps.tile([P, CH], F32, tag="st", bufs=2, name=f"st{c}_{i}")
                e0 = max(i - 4 * c, 0) * P
                nc.tensor.matmul(
                    st[:, e0:CH],
                    kT[:, i * P : (i + 1) * P],
                    qT[:, c * CH + e0 : (c + 1) * CH],
                    start=True,
                    stop=True,
                )
                st_ps[i] = st

            emit_s(0)
            if n_k > 1:
                emit_s(1)

            do_proj(*prv, pieces=False)
            # v natural tiles: PE-transpose vT tiles into v_aug
            for tt in range(4 * c, 4 * c + 4):
                vtr = ps.tile([P, P], BF16, tag="mm", bufs=2, name=f"vtr{tt}")
                nc.tensor.transpose(vtr[:], vT[:, tt * P : (tt + 1) * P], ident_b)
                nc.vector.tensor_copy(v_aug[:, tt, 0:H], vtr[:])
            # next chunk's casts go to the DVE queue ahead of the divides
            if c + 1 < QC:
                emit_casts(c + 1)

            for i in range(n_k):
                if i + 2 < n_k:
                    emit_s(i + 2)  # keep PE fed while ACT does exp(i)
                st = st_ps.pop(i)
                pt = work.tile([P, CH], BF16, tag="pt", name=f"pt{c}_{i}")
                j0 = i - 4 * c  # diag offset (>=0 on the diagonal tile)
                e0 = max(j0, 0) * P
                nc.scalar.activation(
                    pt[:, e0:CH],
                    st[:, e0:CH],
                    mybir.ActivationFunctionType.Exp,
                    scale=SCALE,
                )
                if j0 >= 0:
                    # zero the causal triangle of the diagonal block
                    nc.gpsimd.affine_select(
                        out=pt[:, e0 : e0 + P],
                        in_=pt[:, e0 : e0 + P],
                        compare_op=mybir.AluOpType.is_ge,
                        fill=0.0,
                        base=0,
                        pattern=[[1, P]],
                        channel_multiplier=-1,
                    )
                for s in range(4):
                    if i <= 4 * c + s:
                        nc.tensor.matmul(
                            o_ps[s][:],
                            pt[:, s * P : (s + 1) * P],
                            v_aug[:, i, :],
                            start=(i == 0),
                            stop=(i == 4 * c + s),
                        )
                # q-tile whose accumulation chain just stopped: divide + out
                s = i - 4 * c
                if 0 <= s < 4:
                    qt_idx = 4 * c + s
                    recip = work.tile(
                        [P, 1], F32, tag="recip", bufs=16, name=f"rcp{qt_idx}"
                    )
                    nc.vector.reciprocal(recip[:], o_ps[s][:, H : H + 1])
                    o_sb = work.tile(
                        [P, H], F32, tag="o_sb", bufs=16, name=f"o_sb{qt_idx}"
                    )
                    nc.vector.tensor_scalar_mul(o_sb[:], o_ps[s][:, 0:H], recip[:])
                    nc.sync.dma_start(out[qt_idx * P : (qt_idx + 1) * P, :], o_sb[:])


_NC_CACHE = None


def _get_nc():
    global _NC_CACHE
    if _NC_CACHE is None:
        _NC_CACHE = build_nc()
    return _NC_CACHE


def kernel(**inputs):
    x = np.asarray(inputs["x"], dtype=np.float32)

    def host_reshape(w):
        # [D, H] -> [p, dt, h] (pure layout permutation, dtype preserved)
        w = np.asarray(w, dtype=np.float32)
        return np.ascontiguousarray(w.reshape(DT, P, H).transpose(1, 0, 2))

    wq = host_reshape(inputs["Wq"])
    wk = host_reshape(inputs["Wk"])
    wv = host_reshape(inputs["Wv"])
    assert x.shape == (B, T, D)

    def host_xt(xb):
        # [T, D] -> x^T t-tile slabs [tt, p, dt, u] (layout permutation only)
        return np.ascontiguousarray(
            xb.T.reshape(DT, P, TT, P).transpose(2, 1, 0, 3)
        )

    nc = _get_nc()
    in_maps = [
        {"xt": host_xt(x[b]), "wq": wq, "wk": wk, "wv": wv} for b in range(N_CORES)
    ]
    res = bass_utils.run_bass_kernel_spmd(nc, in_maps, core_ids=list(range(N_CORES)))
    return np.stack([res.results[b]["out"] for b in range(N_CORES)], axis=0)
